# revision 24
# baseline (speedup 1.0000x reference)
"""AttnBlock (GroupNorm + 1-head spatial self-attention + residual) on 8 trn2 cores.

Sharding: B=4 images, 2 cores per image. Each core receives its full image
(GN stats and K/V need all n=4096 positions) and computes the attention rows
for its half of the query positions. Odd cores receive the image rolled by
2048 along n so every core runs the identical SPMD program (attention output
is invariant to a permutation of key positions).

Per core (C=256 split into 2 chunks of 128 partitions):
  GN stats (ACT square-accum + DVE reduces + tiny grouping matmuls) are folded
  into the projection weights: Wq' = Wq*scale_c, bias' = W@shift + b, so x
  feeds every matmul directly (no normalized copy of x is materialized).
  q = Wq'.T@x (cols 0:2048) ; k = Wk'.T@x ; vT = x.T@Wv'
  scoresT[j,i] = k.T q  (transposed: softmax sums land on the matmul K axis)
  e = exp(scoresT/16) on ACT straight from PSUM (no max subtraction: scores
  are ~N(0,1), exp never overflows fp32)
  den[i] = sum_j e[j,i]: strided reduces + one ones-vector matmul
  AV: h_unnorm[c,i] = sum_j vT[j,c] e[j,i] ; O_unnorm = Wo.T @ h_unnorm
  Device returns O_unnorm and den; the host computes
  out = x + O_unnorm/den + bo  (normalization commutes with the 1x1 conv),
  keeping the residual in exact fp32.
All matmuls run as float32r (tf32-style rounded fp32; ~1e-5 rel precision,
1 cycle/row streaming).
"""

import numpy as np

N = 4096  # spatial positions per image
NHALF = 2048  # query positions per core
C = 256
NCHUNK = 2  # channel chunks of 128
P = 128
NG = 32  # groups
GS = 8  # channels per group
EPS = 1e-6
SCALE = float(C) ** -0.5  # 0.0625
NBLK = 4  # i-blocks of 512 per core
BLK = 512
NJC = 32  # j-chunks of 128
QUART = 8  # j-chunks per exp quarter-buffer
DEN_ENGINE = "gpsimd"  # or "vector"

_CACHE = {}


def _build_program():
    import concourse.bacc as bacc
    import concourse.mybir as mybir
    import concourse.tile as tile

    f32 = mybir.dt.float32
    f32r = mybir.dt.float32r
    AF = mybir.ActivationFunctionType
    OP = mybir.AluOpType
    AX = mybir.AxisListType

    nc = bacc.Bacc("TRN2", target_bir_lowering=False)

    # DRAM I/O
    xa_d = nc.dram_tensor("xa", [NCHUNK, P, NHALF], f32r, kind="ExternalInput")
    xb_d = nc.dram_tensor("xb", [NCHUNK, P, NHALF], f32r, kind="ExternalInput")
    wq_d = nc.dram_tensor("wq", [P, NCHUNK, NCHUNK, P], f32r, kind="ExternalInput")
    wk_d = nc.dram_tensor("wk", [P, NCHUNK, NCHUNK, P], f32r, kind="ExternalInput")
    wo_d = nc.dram_tensor("wo", [P, NCHUNK, NCHUNK, P], f32r, kind="ExternalInput")
    wv_d = nc.dram_tensor("wv", [P, NCHUNK, C], f32r, kind="ExternalInput")
    bq_d = nc.dram_tensor("bq", [P, NCHUNK], f32, kind="ExternalInput")
    bk_d = nc.dram_tensor("bk", [P, NCHUNK], f32, kind="ExternalInput")
    bv_d = nc.dram_tensor("bv", [1, C], f32r, kind="ExternalInput")
    gamma_d = nc.dram_tensor("gamma", [P, NCHUNK], f32, kind="ExternalInput")
    beta_d = nc.dram_tensor("beta", [P, NCHUNK], f32, kind="ExternalInput")
    gmat_d = nc.dram_tensor("gmat", [P, 16], f32r, kind="ExternalInput")
    selmat_d = nc.dram_tensor("selmat", [16, P], f32r, kind="ExternalInput")
    out_d = nc.dram_tensor("out", [NCHUNK, P, NHALF], f32, kind="ExternalOutput")
    den_d = nc.dram_tensor("den", [1, NHALF], f32, kind="ExternalOutput")

    with tile.TileContext(nc) as tc:
        den_eng = nc.gpsimd if DEN_ENGINE == "gpsimd" else nc.vector
        with (
            tc.tile_pool(name="res", bufs=1) as res_pool,
            tc.tile_pool(name="big16", bufs=2) as big16_pool,
            tc.tile_pool(name="kpool", bufs=1) as k_pool,
            tc.tile_pool(name="qpool", bufs=1) as q_pool,
            tc.tile_pool(name="vpool", bufs=1) as v_pool,
            tc.tile_pool(name="hpool", bufs=2) as h_pool,
            tc.tile_pool(name="opool", bufs=4) as o_pool,
            tc.tile_pool(name="wpool", bufs=1) as w_pool,
            tc.tile_pool(name="small", bufs=1) as s_pool,
            tc.tile_pool(name="scr", bufs=2) as scr_pool,
            tc.tile_pool(name="ps_s", bufs=2, space="PSUM") as ps_s,
            tc.tile_pool(name="ps_av", bufs=1, space="PSUM") as ps_av,
            tc.tile_pool(name="ps_den", bufs=1, space="PSUM") as ps_den,
            tc.tile_pool(name="ps_misc", bufs=1, space="PSUM") as ps_misc,
        ):
            # ---- loads ----
            xa = res_pool.tile([P, NCHUNK, NHALF], f32r, tag="xa")
            xb = res_pool.tile([P, NCHUNK, NHALF], f32r, tag="xb")
            for a in range(NCHUNK):
                nc.sync.dma_start(
                    xa[:, a, :], xa_d.ap().rearrange("a p n -> p a n")[:, a, :]
                )
                nc.sync.dma_start(
                    xb[:, a, :], xb_d.ap().rearrange("a p n -> p a n")[:, a, :]
                )

            wq = w_pool.tile([P, NCHUNK, NCHUNK, P], f32r, tag="wq")
            nc.sync.dma_start(wq[:], wq_d.ap())
            wk = w_pool.tile([P, NCHUNK, NCHUNK, P], f32r, tag="wk")
            nc.sync.dma_start(wk[:], wk_d.ap())
            wo = w_pool.tile([P, NCHUNK, NCHUNK, P], f32r, tag="wo")
            nc.sync.dma_start(wo[:], wo_d.ap())
            wv = w_pool.tile([P, NCHUNK, C], f32r, tag="wv")
            nc.sync.dma_start(wv[:], wv_d.ap())

            bq = s_pool.tile([P, NCHUNK], f32, tag="bq")
            nc.sync.dma_start(bq[:], bq_d.ap())
            bk = s_pool.tile([P, NCHUNK], f32, tag="bk")
            nc.sync.dma_start(bk[:], bk_d.ap())
            bv = s_pool.tile([1, C], f32r, tag="bv")
            nc.sync.dma_start(bv[:], bv_d.ap())
            gam = s_pool.tile([P, NCHUNK], f32, tag="gam")
            nc.sync.dma_start(gam[:], gamma_d.ap())
            bet = s_pool.tile([P, NCHUNK], f32, tag="bet")
            nc.sync.dma_start(bet[:], beta_d.ap())
            gmat = s_pool.tile([P, 16], f32r, tag="gmat")
            nc.sync.dma_start(gmat[:], gmat_d.ap())
            selmat = s_pool.tile([16, P], f32r, tag="selmat")
            nc.sync.dma_start(selmat[:], selmat_d.ap())

            ones_r = s_pool.tile([1, P], f32r, tag="ones_r")
            nc.gpsimd.memset(ones_r[:].bitcast(f32), 1.0)
            ones_c = s_pool.tile([P, 1], f32r, tag="ones_c")
            nc.gpsimd.memset(ones_c[:].bitcast(f32), 1.0)
            zb = s_pool.tile([P, 1], f32, tag="zb")
            nc.gpsimd.memset(zb[:], 0.0)

            # ---- GroupNorm stats ----
            # st[:, a, 0] = sum_n x[c, n], st[:, a, 1] = sum_n x[c, n]^2
            st = s_pool.tile([P, NCHUNK, 2], f32r, tag="st")
            sqacc = s_pool.tile([P, NCHUNK, 8], f32, tag="sqacc")
            s1a = s_pool.tile([P, NCHUNK, 2], f32, tag="s1a")
            for a in range(NCHUNK):
                for half, xt in ((0, xa), (1, xb)):
                    for s in range(4):
                        scr = scr_pool.tile([P, BLK], f32, tag="scr")
                        nc.scalar.activation(
                            scr[:],
                            xt[:, a, s * BLK : (s + 1) * BLK],
                            AF.Square,
                            bias=zb[:],
                            accum_out=sqacc[:, a, half * 4 + s : half * 4 + s + 1],
                        )
                nc.vector.reduce_sum(s1a[:, a, 0:1], xa[:, a, :], axis=AX.X)
                nc.vector.reduce_sum(s1a[:, a, 1:2], xb[:, a, :], axis=AX.X)
                with nc.allow_low_precision(reason="f32r rounding of fp32 sums"):
                    nc.vector.reduce_sum(st[:, a, 0:1], s1a[:, a, :], axis=AX.X)
                    nc.vector.reduce_sum(st[:, a, 1:2], sqacc[:, a, :], axis=AX.X)

            # group sums: [16, (s1_a0, s2_a0, s1_a1, s2_a1)] — chunk a in col pair 2a
            gst_ps = ps_misc.tile([16, 4], f32, tag="ps_misc")
            for a in range(NCHUNK):
                nc.tensor.matmul(
                    gst_ps[:, 2 * a : 2 * a + 2],
                    gmat[:],
                    st[:, a, :],
                    start=True,
                    stop=True,
                )
            gst = s_pool.tile([16, 4], f32, tag="gst")
            nc.vector.tensor_copy(gst[:], gst_ps[:])

            # grp[:, a, 0] = group mean, grp[:, a, 1] = group rstd
            inv_n = 1.0 / (GS * N)
            mv = s_pool.tile([16, 4], f32, tag="mv")
            nc.vector.tensor_scalar_mul(mv[:], gst[:], inv_n)
            msq = s_pool.tile([16, 2], f32, tag="msq")
            var2 = s_pool.tile([16, 2], f32, tag="var2")
            sd2 = s_pool.tile([16, 2], f32, tag="sd2")
            grp = s_pool.tile([16, NCHUNK, 2], f32r, tag="grp")
            for a in range(NCHUNK):
                nc.vector.tensor_tensor(
                    msq[:, a : a + 1],
                    mv[:, 2 * a : 2 * a + 1],
                    mv[:, 2 * a : 2 * a + 1],
                    op=OP.mult,
                )
                nc.vector.tensor_tensor(
                    var2[:, a : a + 1],
                    mv[:, 2 * a + 1 : 2 * a + 2],
                    msq[:, a : a + 1],
                    op=OP.subtract,
                )
            epst = s_pool.tile([16, 1], f32, tag="epst")
            nc.gpsimd.memset(epst[:], float(EPS))
            nc.scalar.activation(sd2[:], var2[:], AF.Sqrt, bias=epst[:])
            with nc.allow_low_precision(reason="f32r rounding for matmul feed"):
                nc.vector.reciprocal(grp[:, 0, 1:2], sd2[:, 0:1])
                nc.vector.reciprocal(grp[:, 1, 1:2], sd2[:, 1:2])
                nc.vector.tensor_copy(grp[:, 0, 0:1], mv[:, 0:1])
                nc.vector.tensor_copy(grp[:, 1, 0:1], mv[:, 2:3])

            # broadcast group stats to channels: pcs[:, a, 0]=mean_c, [:, a, 1]=rstd_c
            pcs = s_pool.tile([P, NCHUNK, 2], f32, tag="pcs")
            for a in range(NCHUNK):
                pcs_ps = ps_misc.tile([P, 2], f32, tag="ps_misc")
                nc.tensor.matmul(
                    pcs_ps[:], selmat[:], grp[:, a, :], start=True, stop=True
                )
                nc.vector.tensor_copy(pcs[:, a, :], pcs_ps[:])

            # per-channel affine: af[:, a, 0] = gamma*rstd, af[:, a, 1] = beta - mean*gamma*rstd
            af = s_pool.tile([P, NCHUNK, 2], f32, tag="af")
            msc = s_pool.tile([P, NCHUNK], f32, tag="msc")
            for a in range(NCHUNK):
                nc.vector.tensor_tensor(
                    af[:, a, 0:1], gam[:, a : a + 1], pcs[:, a, 1:2], op=OP.mult
                )
                nc.vector.tensor_tensor(
                    msc[:, a : a + 1], pcs[:, a, 0:1], af[:, a, 0:1], op=OP.mult
                )
                nc.vector.tensor_tensor(
                    af[:, a, 1:2], bet[:, a : a + 1], msc[:, a : a + 1], op=OP.subtract
                )
            shf_r = s_pool.tile([P, NCHUNK], f32r, tag="shf_r")
            with nc.allow_low_precision(reason="f32r rounding for matmul feed"):
                for a in range(NCHUNK):
                    nc.vector.tensor_copy(shf_r[:, a : a + 1], af[:, a, 1:2])

            # ---- fold GN into projection weights ----
            # corrected biases first (they need the unscaled weights), then
            # scale the weight rows in place.
            bq2 = s_pool.tile([P, NCHUNK], f32, tag="bq2")
            bk2 = s_pool.tile([P, NCHUNK], f32, tag="bk2")
            for w_t, b_t, b2_t in ((wq, bq, bq2), (wk, bk, bk2)):
                for b in range(NCHUNK):
                    bp = ps_misc.tile([P, 1], f32, tag="ps_misc")
                    nc.tensor.matmul(
                        bp[:],
                        w_t[:, 0, b, :].bitcast(f32),
                        af[:, 0, 1:2],
                        start=True,
                        stop=False,
                    )
                    nc.tensor.matmul(
                        bp[:],
                        w_t[:, 1, b, :].bitcast(f32),
                        af[:, 1, 1:2],
                        start=False,
                        stop=True,
                    )
                    nc.vector.tensor_tensor(
                        b2_t[:, b : b + 1], bp[:], b_t[:, b : b + 1], op=OP.add
                    )
            # v bias row: bvrow = bv + shift @ WvT, broadcast to [P, C]
            vr_ps = ps_misc.tile([1, C], f32, tag="ps_misc")
            nc.tensor.matmul(vr_ps[:], shf_r[:, 0:1], wv[:, 0, :], start=True, stop=False)
            nc.tensor.matmul(vr_ps[:], shf_r[:, 1:2], wv[:, 1, :], start=False, stop=True)
            bvrow = s_pool.tile([1, C], f32r, tag="bvrow")
            with nc.allow_low_precision(reason="f32r rounding for matmul feed"):
                nc.vector.tensor_tensor(bvrow[:], vr_ps[:], bv[:], op=OP.add)
            bvb_ps = ps_misc.tile([P, C], f32, tag="ps_misc")
            nc.tensor.matmul(bvb_ps[:], ones_r[:], bvrow[:], start=True, stop=True)
            bvb = s_pool.tile([P, C], f32, tag="bvb")
            nc.vector.tensor_copy(bvb[:], bvb_ps[:])

            # scale weight rows in place: w[c', :] *= scale[c']
            with nc.allow_low_precision(reason="f32r weights"):
                for a in range(NCHUNK):
                    nc.vector.tensor_scalar_mul(
                        wq[:, a, :, :], wq[:, a, :, :], af[:, a, 0:1]
                    )
                    nc.vector.tensor_scalar_mul(
                        wk[:, a, :, :], wk[:, a, :, :], af[:, a, 0:1]
                    )
                    nc.vector.tensor_scalar_mul(
                        wv[:, a, :], wv[:, a, :], af[:, a, 0:1]
                    )

            vt = v_pool.tile([P, NJC, C + 1], f32r, tag="vt")
            nc.gpsimd.memset(vt[:, :, C : C + 1].bitcast(f32), 1.0)
            k_t = k_pool.tile([P, NCHUNK, N], f32r, tag="k")
            q_t = q_pool.tile([P, NCHUNK, NHALF], f32r, tag="q")

            # ---- projections straight from x ----
            for s in range(8):
                xsrc = xa if s < 4 else xb
                soff = (s % 4) * BLK
                xs0 = xsrc[:, 0, soff : soff + BLK]
                xs1 = xsrc[:, 1, soff : soff + BLK]
                # q projection (first 4 strips = this core's queries)
                if s < 4:
                    for b in range(NCHUNK):
                        qp = ps_s.tile([P, BLK], f32, tag="ps_sp")
                        nc.tensor.matmul(
                            qp[:], wq[:, 0, b, :], xs0, start=True, stop=False
                        )
                        nc.tensor.matmul(
                            qp[:], wq[:, 1, b, :], xs1, start=False, stop=True
                        )
                        nc.vector.tensor_scalar_add(
                            q_t[:, b, s * BLK : (s + 1) * BLK], qp[:], bq2[:, b : b + 1]
                        )
                for b in range(NCHUNK):
                    kp = ps_s.tile([P, BLK], f32, tag="ps_sp")
                    nc.tensor.matmul(kp[:], wk[:, 0, b, :], xs0, start=True, stop=False)
                    nc.tensor.matmul(kp[:], wk[:, 1, b, :], xs1, start=False, stop=True)
                    nc.vector.tensor_scalar_add(
                        k_t[:, b, s * BLK : (s + 1) * BLK], kp[:], bk2[:, b : b + 1]
                    )
                # vT projection: strip s covers j-chunks 4s..4s+3
                for jj in range(4):
                    jc = 4 * s + jj
                    vp = ps_s.tile([P, C], f32, tag="ps_sp")
                    nc.tensor.matmul(
                        vp[:],
                        xs0[:, jj * P : (jj + 1) * P],
                        wv[:, 0, :],
                        start=True,
                        stop=False,
                    )
                    nc.tensor.matmul(
                        vp[:],
                        xs1[:, jj * P : (jj + 1) * P],
                        wv[:, 1, :],
                        start=False,
                        stop=True,
                    )
                    nc.vector.tensor_tensor(vt[:, jc, 0:C], vp[:], bvb[:], op=OP.add)

            # ---- attention blocks ----
            for blk in range(NBLK):
                ib = blk * BLK
                av = ps_av.tile([P, NCHUNK, BLK], f32, tag="ps_av")
                den = ps_den.tile([1, BLK], f32, tag="ps_den")
                for quart in range(NJC // QUART):
                    eq = big16_pool.tile([P, QUART, BLK], f32r, tag="big16")
                    for pair in range(QUART // 2):
                        sp = ps_s.tile([P, 2, BLK], f32, tag="ps_sp")
                        for u in range(2):
                            jc = quart * QUART + pair * 2 + u
                            nc.tensor.matmul(
                                sp[:, u, :],
                                k_t[:, 0, jc * P : (jc + 1) * P],
                                q_t[:, 0, ib : ib + BLK],
                                start=True,
                                stop=False,
                            )
                            nc.tensor.matmul(
                                sp[:, u, :],
                                k_t[:, 1, jc * P : (jc + 1) * P],
                                q_t[:, 1, ib : ib + BLK],
                                start=False,
                                stop=True,
                            )
                        nc.scalar.activation(
                            eq[:, 2 * pair : 2 * pair + 2, :],
                            sp[:],
                            AF.Exp,
                            bias=zb[:],
                            scale=SCALE,
                        )
                    for jj in range(QUART):
                        jc = quart * QUART + jj
                        for m in range(NCHUNK):
                            nc.tensor.matmul(
                                av[:, m, :],
                                vt[:, jc, m * P : (m + 1) * P],
                                eq[:, jj, :],
                                start=(jc == 0),
                                stop=(jc == NJC - 1),
                            )
                        nc.tensor.matmul(
                            den[:],
                            vt[:, jc, C : C + 1],
                            eq[:, jj, :],
                            start=(jc == 0),
                            stop=(jc == NJC - 1),
                        )

                den_sb = o_pool.tile([1, BLK], f32, tag="den_sb")
                nc.vector.tensor_copy(den_sb[:], den[:])
                nc.sync.dma_start(den_d.ap()[:, ib : ib + BLK], den_sb[:])

                # h_unnorm psum -> sbuf, then output projection (unnormalized)
                h_t = h_pool.tile([P, NCHUNK, BLK], f32r, tag="h")
                with nc.allow_low_precision(reason="f32r rounding for matmul feed"):
                    for m in range(NCHUNK):
                        nc.scalar.copy(h_t[:, m, :], av[:, m, :])

                for b in range(NCHUNK):
                    po = ps_misc.tile([P, BLK], f32, tag="ps_misc")
                    nc.tensor.matmul(
                        po[:], wo[:, 0, b, :], h_t[:, 0, :], start=True, stop=False
                    )
                    nc.tensor.matmul(
                        po[:], wo[:, 1, b, :], h_t[:, 1, :], start=False, stop=True
                    )
                    ot = o_pool.tile([P, BLK], f32, tag="o")
                    nc.vector.tensor_copy(ot[:], po[:])
                    nc.sync.dma_start(
                        out_d.ap().rearrange("a p n -> p a n")[:, b, ib : ib + BLK],
                        ot[:],
                    )

    nc.compile()
    return nc


def _prep_shards(x, gamma, beta, Wq, bq, Wk, bk, Wv, bv, Wo, bo):
    xr = np.ascontiguousarray(x, dtype=np.float32).reshape(4, C, N)

    def w4(W):
        # w4[p, a, b, m] = W[b*128+m, a*128+p]
        return np.ascontiguousarray(
            np.asarray(W, np.float32).reshape(NCHUNK, P, NCHUNK, P).transpose(3, 2, 0, 1)
        )

    wv3 = np.ascontiguousarray(
        np.asarray(Wv, np.float32).reshape(C, NCHUNK, P).transpose(2, 1, 0)
    )

    def b2(v):
        return np.ascontiguousarray(np.asarray(v, np.float32).reshape(NCHUNK, P).T)

    gmat = np.zeros((P, 16), np.float32)
    for p in range(P):
        gmat[p, p // GS] = 1.0
    selmat = np.zeros((16, P), np.float32)
    for p in range(P):
        selmat[p // GS, p] = 1.0

    shared = {
        "wq": w4(Wq),
        "wk": w4(Wk),
        "wo": w4(Wo),
        "wv": wv3,
        "bq": b2(bq),
        "bk": b2(bk),
        "bv": np.ascontiguousarray(np.asarray(bv, np.float32).reshape(1, C)),
        "gamma": b2(gamma),
        "beta": b2(beta),
        "gmat": gmat,
        "selmat": selmat,
    }

    in_maps = []
    for core in range(8):
        img = core // 2
        xi = xr[img].reshape(NCHUNK, P, N)
        if core % 2 == 0:
            xa_h, xb_h = xi[:, :, :NHALF], xi[:, :, NHALF:]
        else:
            xa_h, xb_h = xi[:, :, NHALF:], xi[:, :, :NHALF]
        m = dict(shared)
        m["xa"] = np.ascontiguousarray(xa_h)
        m["xb"] = np.ascontiguousarray(xb_h)
        in_maps.append(m)
    return in_maps


def kernel(x, gamma, beta, Wq, bq, Wk, bk, Wv, bv, Wo, bo, _trace=False):
    from concourse.bass_utils import run_bass_kernel_spmd

    if "nc" not in _CACHE:
        _CACHE["nc"] = _build_program()
    nc = _CACHE["nc"]

    in_maps = _prep_shards(x, gamma, beta, Wq, bq, Wk, bk, Wv, bv, Wo, bo)
    res = run_bass_kernel_spmd(nc, in_maps, core_ids=list(range(8)), trace=_trace)
    _CACHE["last_results"] = res

    x_np = np.ascontiguousarray(x, dtype=np.float32).reshape(4, C, N)
    bo_np = np.asarray(bo, np.float32).reshape(C, 1)
    y = np.empty((4, C, N), np.float32)
    for core in range(8):
        o = res.results[core]["out"].reshape(C, NHALF)
        den = res.results[core]["den"].reshape(1, NHALF)
        img = core // 2
        lo, hi = (0, NHALF) if core % 2 == 0 else (NHALF, N)
        y[img, :, lo:hi] = x_np[img, :, lo:hi] + o / den + bo_np
    return y.reshape(4, C, 64, 64)


# revision 25
# speedup vs baseline: 1.0751x; 1.0751x over previous
"""AttnBlock (GroupNorm + 1-head spatial self-attention + residual) on 8 trn2 cores.

Sharding: B=4 images, 2 cores per image. Each core receives its full image
(GN stats and K/V need all n=4096 positions) and computes the attention rows
for its half of the query positions. Odd cores receive the image rolled by
2048 along n so every core runs the identical SPMD program (attention output
is invariant to a permutation of key positions).

Per core (C=256 split into 2 chunks of 128 partitions):
  GN stats (ACT square-accum + DVE reduces + tiny grouping matmuls) are folded
  into the projection weights: Wq' = Wq*scale_c, bias' = W@shift + b, so x
  feeds every matmul directly (no normalized copy of x is materialized).
  q = Wq'.T@x (cols 0:2048) ; k = Wk'.T@x ; vT = x.T@Wv'
  scoresT[j,i] = k.T q  (transposed: softmax sums land on the matmul K axis)
  e = exp(scoresT/16) on ACT straight from PSUM (no max subtraction: scores
  are ~N(0,1), exp never overflows fp32)
  den[i] = sum_j e[j,i]: strided reduces + one ones-vector matmul
  AV: h_unnorm[c,i] = sum_j vT[j,c] e[j,i] ; O_unnorm = Wo.T @ h_unnorm
  Device returns O_unnorm and den; the host computes
  out = x + O_unnorm/den + bo  (normalization commutes with the 1x1 conv),
  keeping the residual in exact fp32.
All matmuls run as float32r (tf32-style rounded fp32; ~1e-5 rel precision,
1 cycle/row streaming).
"""

import numpy as np

N = 4096  # spatial positions per image
NHALF = 2048  # query positions per core
C = 256
NCHUNK = 2  # channel chunks of 128
P = 128
NG = 32  # groups
GS = 8  # channels per group
EPS = 1e-6
SCALE = float(C) ** -0.5  # 0.0625
NBLK = 4  # i-blocks of 512 per core
BLK = 512
NJC = 32  # j-chunks of 128
QUART = 4  # j-chunks per exp quarter-buffer
DEN_ENGINE = "gpsimd"  # or "vector"

_CACHE = {}


def _build_program():
    import concourse.bacc as bacc
    import concourse.mybir as mybir
    import concourse.tile as tile

    f32 = mybir.dt.float32
    f32r = mybir.dt.float32r
    AF = mybir.ActivationFunctionType
    OP = mybir.AluOpType
    AX = mybir.AxisListType

    nc = bacc.Bacc("TRN2", target_bir_lowering=False)

    # DRAM I/O
    xa_d = nc.dram_tensor("xa", [NCHUNK, P, NHALF], f32r, kind="ExternalInput")
    xb_d = nc.dram_tensor("xb", [NCHUNK, P, NHALF], f32r, kind="ExternalInput")
    wq_d = nc.dram_tensor("wq", [P, NCHUNK, NCHUNK, P], f32r, kind="ExternalInput")
    wk_d = nc.dram_tensor("wk", [P, NCHUNK, NCHUNK, P], f32r, kind="ExternalInput")
    wo_d = nc.dram_tensor("wo", [P, NCHUNK, NCHUNK, P], f32r, kind="ExternalInput")
    wv_d = nc.dram_tensor("wv", [P, NCHUNK, C], f32r, kind="ExternalInput")
    bq_d = nc.dram_tensor("bq", [P, NCHUNK], f32, kind="ExternalInput")
    bk_d = nc.dram_tensor("bk", [P, NCHUNK], f32, kind="ExternalInput")
    bv_d = nc.dram_tensor("bv", [1, C], f32r, kind="ExternalInput")
    gamma_d = nc.dram_tensor("gamma", [P, NCHUNK], f32, kind="ExternalInput")
    beta_d = nc.dram_tensor("beta", [P, NCHUNK], f32, kind="ExternalInput")
    gmat_d = nc.dram_tensor("gmat", [P, 16], f32r, kind="ExternalInput")
    selmat_d = nc.dram_tensor("selmat", [16, P], f32r, kind="ExternalInput")
    out_d = nc.dram_tensor("out", [NCHUNK, P, NHALF], f32, kind="ExternalOutput")
    den_d = nc.dram_tensor("den", [1, NHALF], f32, kind="ExternalOutput")

    with tile.TileContext(nc) as tc:
        den_eng = nc.gpsimd if DEN_ENGINE == "gpsimd" else nc.vector
        with (
            tc.tile_pool(name="res", bufs=1) as res_pool,
            tc.tile_pool(name="big16", bufs=3) as big16_pool,
            tc.tile_pool(name="kpool", bufs=1) as k_pool,
            tc.tile_pool(name="qpool", bufs=1) as q_pool,
            tc.tile_pool(name="vpool", bufs=1) as v_pool,
            tc.tile_pool(name="hpool", bufs=2) as h_pool,
            tc.tile_pool(name="opool", bufs=3) as o_pool,
            tc.tile_pool(name="wpool", bufs=1) as w_pool,
            tc.tile_pool(name="small", bufs=1) as s_pool,
            tc.tile_pool(name="scr", bufs=2) as scr_pool,
            tc.tile_pool(name="ps_s", bufs=2, space="PSUM") as ps_s,
            tc.tile_pool(name="ps_av", bufs=1, space="PSUM") as ps_av,
            tc.tile_pool(name="ps_misc", bufs=2, space="PSUM") as ps_misc,
        ):
            # ---- loads ----
            xa = res_pool.tile([P, NCHUNK, NHALF], f32r, tag="xa")
            xb = res_pool.tile([P, NCHUNK, NHALF], f32r, tag="xb")
            for a in range(NCHUNK):
                nc.sync.dma_start(
                    xa[:, a, :], xa_d.ap().rearrange("a p n -> p a n")[:, a, :]
                )
                nc.sync.dma_start(
                    xb[:, a, :], xb_d.ap().rearrange("a p n -> p a n")[:, a, :]
                )

            wq = w_pool.tile([P, NCHUNK, NCHUNK, P], f32r, tag="wq")
            nc.sync.dma_start(wq[:], wq_d.ap())
            wk = w_pool.tile([P, NCHUNK, NCHUNK, P], f32r, tag="wk")
            nc.sync.dma_start(wk[:], wk_d.ap())
            wo = w_pool.tile([P, NCHUNK, NCHUNK, P], f32r, tag="wo")
            nc.sync.dma_start(wo[:], wo_d.ap())
            wv = w_pool.tile([P, NCHUNK, C], f32r, tag="wv")
            nc.sync.dma_start(wv[:], wv_d.ap())

            bq = s_pool.tile([P, NCHUNK], f32, tag="bq")
            nc.sync.dma_start(bq[:], bq_d.ap())
            bk = s_pool.tile([P, NCHUNK], f32, tag="bk")
            nc.sync.dma_start(bk[:], bk_d.ap())
            bv = s_pool.tile([1, C], f32r, tag="bv")
            nc.sync.dma_start(bv[:], bv_d.ap())
            gam = s_pool.tile([P, NCHUNK], f32, tag="gam")
            nc.sync.dma_start(gam[:], gamma_d.ap())
            bet = s_pool.tile([P, NCHUNK], f32, tag="bet")
            nc.sync.dma_start(bet[:], beta_d.ap())
            gmat = s_pool.tile([P, 16], f32r, tag="gmat")
            nc.sync.dma_start(gmat[:], gmat_d.ap())
            selmat = s_pool.tile([16, P], f32r, tag="selmat")
            nc.sync.dma_start(selmat[:], selmat_d.ap())

            ones_r = s_pool.tile([1, P], f32r, tag="ones_r")
            nc.gpsimd.memset(ones_r[:].bitcast(f32), 1.0)
            ones_c = s_pool.tile([P, 1], f32r, tag="ones_c")
            nc.gpsimd.memset(ones_c[:].bitcast(f32), 1.0)
            zb = s_pool.tile([P, 1], f32, tag="zb")
            nc.gpsimd.memset(zb[:], 0.0)

            # ---- GroupNorm stats ----
            # st[:, a, 0] = sum_n x[c, n], st[:, a, 1] = sum_n x[c, n]^2
            st = s_pool.tile([P, NCHUNK, 2], f32r, tag="st")
            sqacc = s_pool.tile([P, NCHUNK, 8], f32, tag="sqacc")
            s1a = s_pool.tile([P, NCHUNK, 2], f32, tag="s1a")
            for a in range(NCHUNK):
                for half, xt in ((0, xa), (1, xb)):
                    for s in range(4):
                        scr = scr_pool.tile([P, BLK], f32, tag="scr")
                        nc.scalar.activation(
                            scr[:],
                            xt[:, a, s * BLK : (s + 1) * BLK],
                            AF.Square,
                            bias=zb[:],
                            accum_out=sqacc[:, a, half * 4 + s : half * 4 + s + 1],
                        )
                nc.vector.reduce_sum(s1a[:, a, 0:1], xa[:, a, :], axis=AX.X)
                nc.vector.reduce_sum(s1a[:, a, 1:2], xb[:, a, :], axis=AX.X)
                with nc.allow_low_precision(reason="f32r rounding of fp32 sums"):
                    nc.vector.reduce_sum(st[:, a, 0:1], s1a[:, a, :], axis=AX.X)
                    nc.vector.reduce_sum(st[:, a, 1:2], sqacc[:, a, :], axis=AX.X)

            # group sums: [16, (s1_a0, s2_a0, s1_a1, s2_a1)] — chunk a in col pair 2a
            gst_ps = ps_misc.tile([16, 4], f32, tag="ps_misc")
            for a in range(NCHUNK):
                nc.tensor.matmul(
                    gst_ps[:, 2 * a : 2 * a + 2],
                    gmat[:],
                    st[:, a, :],
                    start=True,
                    stop=True,
                )
            gst = s_pool.tile([16, 4], f32, tag="gst")
            nc.vector.tensor_copy(gst[:], gst_ps[:])

            # grp[:, a, 0] = group mean, grp[:, a, 1] = group rstd
            inv_n = 1.0 / (GS * N)
            mv = s_pool.tile([16, 4], f32, tag="mv")
            nc.vector.tensor_scalar_mul(mv[:], gst[:], inv_n)
            msq = s_pool.tile([16, 2], f32, tag="msq")
            var2 = s_pool.tile([16, 2], f32, tag="var2")
            sd2 = s_pool.tile([16, 2], f32, tag="sd2")
            grp = s_pool.tile([16, NCHUNK, 2], f32r, tag="grp")
            for a in range(NCHUNK):
                nc.vector.tensor_tensor(
                    msq[:, a : a + 1],
                    mv[:, 2 * a : 2 * a + 1],
                    mv[:, 2 * a : 2 * a + 1],
                    op=OP.mult,
                )
                nc.vector.tensor_tensor(
                    var2[:, a : a + 1],
                    mv[:, 2 * a + 1 : 2 * a + 2],
                    msq[:, a : a + 1],
                    op=OP.subtract,
                )
            epst = s_pool.tile([16, 1], f32, tag="epst")
            nc.gpsimd.memset(epst[:], float(EPS))
            nc.scalar.activation(sd2[:], var2[:], AF.Sqrt, bias=epst[:])
            with nc.allow_low_precision(reason="f32r rounding for matmul feed"):
                nc.vector.reciprocal(grp[:, 0, 1:2], sd2[:, 0:1])
                nc.vector.reciprocal(grp[:, 1, 1:2], sd2[:, 1:2])
                nc.vector.tensor_copy(grp[:, 0, 0:1], mv[:, 0:1])
                nc.vector.tensor_copy(grp[:, 1, 0:1], mv[:, 2:3])

            # broadcast group stats to channels: pcs[:, a, 0]=mean_c, [:, a, 1]=rstd_c
            pcs = s_pool.tile([P, NCHUNK, 2], f32, tag="pcs")
            for a in range(NCHUNK):
                pcs_ps = ps_misc.tile([P, 2], f32, tag="ps_misc")
                nc.tensor.matmul(
                    pcs_ps[:], selmat[:], grp[:, a, :], start=True, stop=True
                )
                nc.vector.tensor_copy(pcs[:, a, :], pcs_ps[:])

            # per-channel affine: af[:, a, 0] = gamma*rstd, af[:, a, 1] = beta - mean*gamma*rstd
            af = s_pool.tile([P, NCHUNK, 2], f32, tag="af")
            msc = s_pool.tile([P, NCHUNK], f32, tag="msc")
            for a in range(NCHUNK):
                nc.vector.tensor_tensor(
                    af[:, a, 0:1], gam[:, a : a + 1], pcs[:, a, 1:2], op=OP.mult
                )
                nc.vector.tensor_tensor(
                    msc[:, a : a + 1], pcs[:, a, 0:1], af[:, a, 0:1], op=OP.mult
                )
                nc.vector.tensor_tensor(
                    af[:, a, 1:2], bet[:, a : a + 1], msc[:, a : a + 1], op=OP.subtract
                )
            shf_r = s_pool.tile([P, NCHUNK], f32r, tag="shf_r")
            with nc.allow_low_precision(reason="f32r rounding for matmul feed"):
                for a in range(NCHUNK):
                    nc.vector.tensor_copy(shf_r[:, a : a + 1], af[:, a, 1:2])

            # ---- fold GN into projection weights ----
            # corrected biases first (they need the unscaled weights), then
            # scale the weight rows in place.
            bq2 = s_pool.tile([P, NCHUNK], f32, tag="bq2")
            bk2 = s_pool.tile([P, NCHUNK], f32, tag="bk2")
            for w_t, b_t, b2_t in ((wq, bq, bq2), (wk, bk, bk2)):
                for b in range(NCHUNK):
                    bp = ps_misc.tile([P, 1], f32, tag="ps_misc")
                    nc.tensor.matmul(
                        bp[:],
                        w_t[:, 0, b, :].bitcast(f32),
                        af[:, 0, 1:2],
                        start=True,
                        stop=False,
                    )
                    nc.tensor.matmul(
                        bp[:],
                        w_t[:, 1, b, :].bitcast(f32),
                        af[:, 1, 1:2],
                        start=False,
                        stop=True,
                    )
                    nc.vector.tensor_tensor(
                        b2_t[:, b : b + 1], bp[:], b_t[:, b : b + 1], op=OP.add
                    )
            # v bias row: bvrow = bv + shift @ WvT, broadcast to [P, C]
            vr_ps = ps_misc.tile([1, C], f32, tag="ps_misc")
            nc.tensor.matmul(vr_ps[:], shf_r[:, 0:1], wv[:, 0, :], start=True, stop=False)
            nc.tensor.matmul(vr_ps[:], shf_r[:, 1:2], wv[:, 1, :], start=False, stop=True)
            bvrow = s_pool.tile([1, C], f32r, tag="bvrow")
            with nc.allow_low_precision(reason="f32r rounding for matmul feed"):
                nc.vector.tensor_tensor(bvrow[:], vr_ps[:], bv[:], op=OP.add)
            bvb_ps = ps_misc.tile([P, C], f32, tag="ps_misc")
            nc.tensor.matmul(bvb_ps[:], ones_r[:], bvrow[:], start=True, stop=True)
            bvb = s_pool.tile([P, C], f32, tag="bvb")
            nc.vector.tensor_copy(bvb[:], bvb_ps[:])

            # scale weight rows in place: w[c', :] *= scale[c']
            with nc.allow_low_precision(reason="f32r weights"):
                for a in range(NCHUNK):
                    nc.vector.tensor_scalar_mul(
                        wq[:, a, :, :], wq[:, a, :, :], af[:, a, 0:1]
                    )
                    nc.vector.tensor_scalar_mul(
                        wk[:, a, :, :], wk[:, a, :, :], af[:, a, 0:1]
                    )
                    nc.vector.tensor_scalar_mul(
                        wv[:, a, :], wv[:, a, :], af[:, a, 0:1]
                    )

            vt = v_pool.tile([P, NJC, C], f32r, tag="vt")
            k_t = k_pool.tile([P, NCHUNK, N], f32r, tag="k")
            q_t = q_pool.tile([P, NCHUNK, NHALF], f32r, tag="q")

            # ---- projections straight from x ----
            for s in range(8):
                xsrc = xa if s < 4 else xb
                soff = (s % 4) * BLK
                xs0 = xsrc[:, 0, soff : soff + BLK]
                xs1 = xsrc[:, 1, soff : soff + BLK]
                # q projection (first 4 strips = this core's queries)
                if s < 4:
                    for b in range(NCHUNK):
                        qp = ps_s.tile([P, BLK], f32, tag="ps_sp")
                        nc.tensor.matmul(
                            qp[:], wq[:, 0, b, :], xs0, start=True, stop=False
                        )
                        nc.tensor.matmul(
                            qp[:], wq[:, 1, b, :], xs1, start=False, stop=True
                        )
                        nc.vector.tensor_scalar_add(
                            q_t[:, b, s * BLK : (s + 1) * BLK], qp[:], bq2[:, b : b + 1]
                        )
                for b in range(NCHUNK):
                    kp = ps_s.tile([P, BLK], f32, tag="ps_sp")
                    nc.tensor.matmul(kp[:], wk[:, 0, b, :], xs0, start=True, stop=False)
                    nc.tensor.matmul(kp[:], wk[:, 1, b, :], xs1, start=False, stop=True)
                    nc.vector.tensor_scalar_add(
                        k_t[:, b, s * BLK : (s + 1) * BLK], kp[:], bk2[:, b : b + 1]
                    )
                # vT projection: strip s covers j-chunks 4s..4s+3
                for jj in range(4):
                    jc = 4 * s + jj
                    vp = ps_s.tile([P, C], f32, tag="ps_sp")
                    nc.tensor.matmul(
                        vp[:],
                        xs0[:, jj * P : (jj + 1) * P],
                        wv[:, 0, :],
                        start=True,
                        stop=False,
                    )
                    nc.tensor.matmul(
                        vp[:],
                        xs1[:, jj * P : (jj + 1) * P],
                        wv[:, 1, :],
                        start=False,
                        stop=True,
                    )
                    nc.vector.tensor_tensor(vt[:, jc, :], vp[:], bvb[:], op=OP.add)

            # ---- attention blocks ----
            # den partial accumulators: dpA fed by DVE adds (eq rows 0,1 of
            # each quarter), dpB by GpSimd adds (rows 2,3); merged at the end.
            dpA = s_pool.tile([P, NBLK, BLK], f32, tag="dpA")
            dpB = s_pool.tile([P, NBLK, BLK], f32, tag="dpB")
            for blk in range(NBLK):
                ib = blk * BLK
                av = ps_av.tile([P, NCHUNK, BLK], f32, tag="ps_av")
                for quart in range(NJC // QUART):
                    eq = big16_pool.tile([P, QUART, BLK], f32r, tag="big16")
                    for pair in range(QUART // 2):
                        sp = ps_s.tile([P, 2, BLK], f32, tag="ps_sp")
                        for u in range(2):
                            jc = quart * QUART + pair * 2 + u
                            nc.tensor.matmul(
                                sp[:, u, :],
                                k_t[:, 0, jc * P : (jc + 1) * P],
                                q_t[:, 0, ib : ib + BLK],
                                start=True,
                                stop=False,
                            )
                            nc.tensor.matmul(
                                sp[:, u, :],
                                k_t[:, 1, jc * P : (jc + 1) * P],
                                q_t[:, 1, ib : ib + BLK],
                                start=False,
                                stop=True,
                            )
                        nc.scalar.activation(
                            eq[:, 2 * pair : 2 * pair + 2, :],
                            sp[:],
                            AF.Exp,
                            bias=zb[:],
                            scale=SCALE,
                        )
                    for jj in range(QUART):
                        jc = quart * QUART + jj
                        for m in range(NCHUNK):
                            nc.tensor.matmul(
                                av[:, m, :],
                                vt[:, jc, m * P : (m + 1) * P],
                                eq[:, jj, :],
                                start=(jc == 0),
                                stop=(jc == NJC - 1),
                            )
                    # denominator partials (contiguous adds, split DVE/GpSimd)
                    if quart == 0:
                        nc.vector.tensor_tensor(
                            dpA[:, blk, :], eq[:, 0, :], eq[:, 1, :], op=OP.add
                        )
                        nc.gpsimd.tensor_tensor(
                            dpB[:, blk, :], eq[:, 2, :], eq[:, 3, :], op=OP.add
                        )
                    else:
                        t0 = scr_pool.tile([P, BLK], f32, tag="t0")
                        nc.vector.tensor_tensor(
                            t0[:], eq[:, 0, :], eq[:, 1, :], op=OP.add
                        )
                        nc.vector.tensor_tensor(
                            dpA[:, blk, :], dpA[:, blk, :], t0[:], op=OP.add
                        )
                        t1 = scr_pool.tile([P, BLK], f32, tag="t1")
                        nc.gpsimd.tensor_tensor(
                            t1[:], eq[:, 2, :], eq[:, 3, :], op=OP.add
                        )
                        nc.gpsimd.tensor_tensor(
                            dpB[:, blk, :], dpB[:, blk, :], t1[:], op=OP.add
                        )

                # h_unnorm psum -> sbuf, then output projection (unnormalized)
                h_t = h_pool.tile([P, NCHUNK, BLK], f32r, tag="h")
                with nc.allow_low_precision(reason="f32r rounding for matmul feed"):
                    for m in range(NCHUNK):
                        nc.scalar.copy(h_t[:, m, :], av[:, m, :])

                for b in range(NCHUNK):
                    po = ps_misc.tile([P, BLK], f32, tag="ps_misc")
                    nc.tensor.matmul(
                        po[:], wo[:, 0, b, :], h_t[:, 0, :], start=True, stop=False
                    )
                    nc.tensor.matmul(
                        po[:], wo[:, 1, b, :], h_t[:, 1, :], start=False, stop=True
                    )
                    ot = o_pool.tile([P, BLK], f32, tag="o")
                    nc.vector.tensor_copy(ot[:], po[:])
                    nc.sync.dma_start(
                        out_d.ap().rearrange("a p n -> p a n")[:, b, ib : ib + BLK],
                        ot[:],
                    )

            # ---- denominator tail: merge partials, cross-partition sum, DMA ----
            dpm = s_pool.tile([P, BLK], f32r, tag="dpm")
            for blk in range(NBLK):
                with nc.allow_low_precision(reason="f32r for ones matmul"):
                    nc.vector.tensor_tensor(
                        dpm[:], dpA[:, blk, :], dpB[:, blk, :], op=OP.add
                    )
                den_ps = ps_misc.tile([1, BLK], f32, tag="ps_misc")
                nc.tensor.matmul(den_ps[:], ones_c[:], dpm[:], start=True, stop=True)
                den_sb = o_pool.tile([1, BLK], f32, tag="den_sb")
                nc.scalar.copy(den_sb[:], den_ps[:])
                nc.sync.dma_start(den_d.ap()[:, blk * BLK : (blk + 1) * BLK], den_sb[:])

    nc.compile()
    return nc


def _prep_shards(x, gamma, beta, Wq, bq, Wk, bk, Wv, bv, Wo, bo):
    xr = np.ascontiguousarray(x, dtype=np.float32).reshape(4, C, N)

    def w4(W):
        # w4[p, a, b, m] = W[b*128+m, a*128+p]
        return np.ascontiguousarray(
            np.asarray(W, np.float32).reshape(NCHUNK, P, NCHUNK, P).transpose(3, 2, 0, 1)
        )

    wv3 = np.ascontiguousarray(
        np.asarray(Wv, np.float32).reshape(C, NCHUNK, P).transpose(2, 1, 0)
    )

    def b2(v):
        return np.ascontiguousarray(np.asarray(v, np.float32).reshape(NCHUNK, P).T)

    gmat = np.zeros((P, 16), np.float32)
    for p in range(P):
        gmat[p, p // GS] = 1.0
    selmat = np.zeros((16, P), np.float32)
    for p in range(P):
        selmat[p // GS, p] = 1.0

    shared = {
        "wq": w4(Wq),
        "wk": w4(Wk),
        "wo": w4(Wo),
        "wv": wv3,
        "bq": b2(bq),
        "bk": b2(bk),
        "bv": np.ascontiguousarray(np.asarray(bv, np.float32).reshape(1, C)),
        "gamma": b2(gamma),
        "beta": b2(beta),
        "gmat": gmat,
        "selmat": selmat,
    }

    in_maps = []
    for core in range(8):
        img = core // 2
        xi = xr[img].reshape(NCHUNK, P, N)
        if core % 2 == 0:
            xa_h, xb_h = xi[:, :, :NHALF], xi[:, :, NHALF:]
        else:
            xa_h, xb_h = xi[:, :, NHALF:], xi[:, :, :NHALF]
        m = dict(shared)
        m["xa"] = np.ascontiguousarray(xa_h)
        m["xb"] = np.ascontiguousarray(xb_h)
        in_maps.append(m)
    return in_maps


def kernel(x, gamma, beta, Wq, bq, Wk, bk, Wv, bv, Wo, bo, _trace=False):
    from concourse.bass_utils import run_bass_kernel_spmd

    if "nc" not in _CACHE:
        _CACHE["nc"] = _build_program()
    nc = _CACHE["nc"]

    in_maps = _prep_shards(x, gamma, beta, Wq, bq, Wk, bk, Wv, bv, Wo, bo)
    res = run_bass_kernel_spmd(nc, in_maps, core_ids=list(range(8)), trace=_trace)
    _CACHE["last_results"] = res

    x_np = np.ascontiguousarray(x, dtype=np.float32).reshape(4, C, N)
    bo_np = np.asarray(bo, np.float32).reshape(C, 1)
    y = np.empty((4, C, N), np.float32)
    for core in range(8):
        o = res.results[core]["out"].reshape(C, NHALF)
        den = res.results[core]["den"].reshape(1, NHALF)
        img = core // 2
        lo, hi = (0, NHALF) if core % 2 == 0 else (NHALF, N)
        y[img, :, lo:hi] = x_np[img, :, lo:hi] + o / den + bo_np
    return y.reshape(4, C, 64, 64)


# revision 26
# speedup vs baseline: 1.2049x; 1.1207x over previous
"""AttnBlock (GroupNorm + 1-head spatial self-attention + residual) on 8 trn2 cores.

Sharding: B=4 images, 2 cores per image. Each core receives its full image
(GN stats and K/V need all n=4096 positions) and computes the attention rows
for its half of the query positions. Odd cores receive the image rolled by
2048 along n so every core runs the identical SPMD program (attention output
is invariant to a permutation of key positions).

Per core (C=256 split into 2 chunks of 128 partitions):
  GN stats (ACT square-accum + DVE reduces + tiny grouping matmuls) are folded
  into the projection weights: Wq' = Wq*scale_c, bias' = W@shift + b, so x
  feeds every matmul directly (no normalized copy of x is materialized).
  q = Wq'.T@x (cols 0:2048) ; k = Wk'.T@x ; vT = x.T@Wv'
  scoresT[j,i] = k.T q  (transposed: softmax sums land on the matmul K axis)
  e = exp(scoresT/16) on ACT straight from PSUM (no max subtraction: scores
  are ~N(0,1), exp never overflows fp32)
  den[i] = sum_j e[j,i]: strided reduces + one ones-vector matmul
  AV: h_unnorm[c,i] = sum_j vT[j,c] e[j,i] ; O_unnorm = Wo.T @ h_unnorm
  Device returns O_unnorm and den; the host computes
  out = x + O_unnorm/den + bo  (normalization commutes with the 1x1 conv),
  keeping the residual in exact fp32.
All matmuls run as float32r (tf32-style rounded fp32; ~1e-5 rel precision,
1 cycle/row streaming).
"""

import numpy as np

N = 4096  # spatial positions per image
NHALF = 2048  # query positions per core
C = 256
NCHUNK = 2  # channel chunks of 128
P = 128
NG = 32  # groups
GS = 8  # channels per group
EPS = 1e-6
SCALE = float(C) ** -0.5  # 0.0625
NBLK = 4  # i-blocks of 512 per core
BLK = 512
NJC = 32  # j-chunks of 128
QUART = 4  # j-chunks per exp quarter-buffer
DEN_ENGINE = "gpsimd"  # or "vector"

_CACHE = {}


def _build_program():
    import concourse.bacc as bacc
    import concourse.mybir as mybir
    import concourse.tile as tile

    f32 = mybir.dt.float32
    f32r = mybir.dt.float32r
    AF = mybir.ActivationFunctionType
    OP = mybir.AluOpType
    AX = mybir.AxisListType

    nc = bacc.Bacc("TRN2", target_bir_lowering=False)

    # DRAM I/O
    xa_d = nc.dram_tensor("xa", [NCHUNK, P, NHALF], f32r, kind="ExternalInput")
    xb_d = nc.dram_tensor("xb", [NCHUNK, P, NHALF], f32r, kind="ExternalInput")
    wq_d = nc.dram_tensor("wq", [P, NCHUNK, NCHUNK, P], f32r, kind="ExternalInput")
    wk_d = nc.dram_tensor("wk", [P, NCHUNK, NCHUNK, P], f32r, kind="ExternalInput")
    wo_d = nc.dram_tensor("wo", [P, NCHUNK, NCHUNK, P], f32r, kind="ExternalInput")
    wv_d = nc.dram_tensor("wv", [P, NCHUNK, C], f32r, kind="ExternalInput")
    bq_d = nc.dram_tensor("bq", [P, NCHUNK], f32, kind="ExternalInput")
    bk_d = nc.dram_tensor("bk", [P, NCHUNK], f32, kind="ExternalInput")
    bv_d = nc.dram_tensor("bv", [1, C], f32r, kind="ExternalInput")
    gamma_d = nc.dram_tensor("gamma", [P, NCHUNK], f32, kind="ExternalInput")
    beta_d = nc.dram_tensor("beta", [P, NCHUNK], f32, kind="ExternalInput")
    gmat_d = nc.dram_tensor("gmat", [P, 16], f32r, kind="ExternalInput")
    selmat_d = nc.dram_tensor("selmat", [16, P], f32r, kind="ExternalInput")
    out_d = nc.dram_tensor("out", [NCHUNK, P, NHALF], f32, kind="ExternalOutput")
    den_d = nc.dram_tensor("den", [1, NHALF], f32, kind="ExternalOutput")

    with tile.TileContext(nc) as tc:
        den_eng = nc.gpsimd if DEN_ENGINE == "gpsimd" else nc.vector
        with (
            tc.tile_pool(name="res", bufs=1) as res_pool,
            tc.tile_pool(name="big16", bufs=3) as big16_pool,
            tc.tile_pool(name="kpool", bufs=1) as k_pool,
            tc.tile_pool(name="qpool", bufs=1) as q_pool,
            tc.tile_pool(name="vpool", bufs=1) as v_pool,
            tc.tile_pool(name="hpool", bufs=2) as h_pool,
            tc.tile_pool(name="opool", bufs=3) as o_pool,
            tc.tile_pool(name="wpool", bufs=1) as w_pool,
            tc.tile_pool(name="small", bufs=1) as s_pool,
            tc.tile_pool(name="scr", bufs=2) as scr_pool,
            tc.tile_pool(name="ps_s", bufs=2, space="PSUM") as ps_s,
            tc.tile_pool(name="ps_av", bufs=1, space="PSUM") as ps_av,
            tc.tile_pool(name="ps_misc", bufs=2, space="PSUM") as ps_misc,
        ):
            # ---- loads ----
            xa = res_pool.tile([P, NCHUNK, NHALF], f32r, tag="xa")
            xb = res_pool.tile([P, NCHUNK, NHALF], f32r, tag="xb")
            for a in range(NCHUNK):
                nc.sync.dma_start(
                    xa[:, a, :], xa_d.ap().rearrange("a p n -> p a n")[:, a, :]
                )
                nc.sync.dma_start(
                    xb[:, a, :], xb_d.ap().rearrange("a p n -> p a n")[:, a, :]
                )

            wq = w_pool.tile([P, NCHUNK, NCHUNK, P], f32r, tag="wq")
            nc.sync.dma_start(wq[:], wq_d.ap())
            wk = w_pool.tile([P, NCHUNK, NCHUNK, P], f32r, tag="wk")
            nc.sync.dma_start(wk[:], wk_d.ap())
            wo = w_pool.tile([P, NCHUNK, NCHUNK, P], f32r, tag="wo")
            nc.sync.dma_start(wo[:], wo_d.ap())
            wv = w_pool.tile([P, NCHUNK, C], f32r, tag="wv")
            nc.sync.dma_start(wv[:], wv_d.ap())

            bq = s_pool.tile([P, NCHUNK], f32, tag="bq")
            nc.sync.dma_start(bq[:], bq_d.ap())
            bk = s_pool.tile([P, NCHUNK], f32, tag="bk")
            nc.sync.dma_start(bk[:], bk_d.ap())
            bv = s_pool.tile([1, C], f32r, tag="bv")
            nc.sync.dma_start(bv[:], bv_d.ap())
            gam = s_pool.tile([P, NCHUNK], f32, tag="gam")
            nc.sync.dma_start(gam[:], gamma_d.ap())
            bet = s_pool.tile([P, NCHUNK], f32, tag="bet")
            nc.sync.dma_start(bet[:], beta_d.ap())
            gmat = s_pool.tile([P, 16], f32r, tag="gmat")
            nc.sync.dma_start(gmat[:], gmat_d.ap())
            selmat = s_pool.tile([16, P], f32r, tag="selmat")
            nc.sync.dma_start(selmat[:], selmat_d.ap())

            ones_r = s_pool.tile([1, P], f32r, tag="ones_r")
            nc.gpsimd.memset(ones_r[:].bitcast(f32), 1.0)
            ones_c = s_pool.tile([P, 1], f32r, tag="ones_c")
            nc.gpsimd.memset(ones_c[:].bitcast(f32), 1.0)
            zb = s_pool.tile([P, 1], f32, tag="zb")
            nc.gpsimd.memset(zb[:], 0.0)

            # ---- GroupNorm stats ----
            # st[:, a, 0] = sum_n x[c, n], st[:, a, 1] = sum_n x[c, n]^2
            st = s_pool.tile([P, NCHUNK, 2], f32r, tag="st")
            sqacc = s_pool.tile([P, NCHUNK, 8], f32, tag="sqacc")
            s1a = s_pool.tile([P, NCHUNK, 2], f32, tag="s1a")
            for a in range(NCHUNK):
                for half, xt in ((0, xa), (1, xb)):
                    for s in range(4):
                        scr = scr_pool.tile([P, BLK], f32, tag="scr")
                        nc.scalar.activation(
                            scr[:],
                            xt[:, a, s * BLK : (s + 1) * BLK],
                            AF.Square,
                            bias=zb[:],
                            accum_out=sqacc[:, a, half * 4 + s : half * 4 + s + 1],
                        )
                nc.vector.reduce_sum(s1a[:, a, 0:1], xa[:, a, :], axis=AX.X)
                nc.vector.reduce_sum(s1a[:, a, 1:2], xb[:, a, :], axis=AX.X)
                with nc.allow_low_precision(reason="f32r rounding of fp32 sums"):
                    nc.vector.reduce_sum(st[:, a, 0:1], s1a[:, a, :], axis=AX.X)
                    nc.vector.reduce_sum(st[:, a, 1:2], sqacc[:, a, :], axis=AX.X)

            # group sums: [16, (s1_a0, s2_a0, s1_a1, s2_a1)] — chunk a in col pair 2a
            gst_ps = ps_misc.tile([16, 4], f32, tag="ps_misc")
            for a in range(NCHUNK):
                nc.tensor.matmul(
                    gst_ps[:, 2 * a : 2 * a + 2],
                    gmat[:],
                    st[:, a, :],
                    start=True,
                    stop=True,
                )
            gst = s_pool.tile([16, 4], f32, tag="gst")
            nc.vector.tensor_copy(gst[:], gst_ps[:])

            # grp[:, a, 0] = group mean, grp[:, a, 1] = group rstd
            inv_n = 1.0 / (GS * N)
            mv = s_pool.tile([16, 4], f32, tag="mv")
            nc.vector.tensor_scalar_mul(mv[:], gst[:], inv_n)
            msq = s_pool.tile([16, 2], f32, tag="msq")
            var2 = s_pool.tile([16, 2], f32, tag="var2")
            sd2 = s_pool.tile([16, 2], f32, tag="sd2")
            grp = s_pool.tile([16, NCHUNK, 2], f32r, tag="grp")
            for a in range(NCHUNK):
                nc.vector.tensor_tensor(
                    msq[:, a : a + 1],
                    mv[:, 2 * a : 2 * a + 1],
                    mv[:, 2 * a : 2 * a + 1],
                    op=OP.mult,
                )
                nc.vector.tensor_tensor(
                    var2[:, a : a + 1],
                    mv[:, 2 * a + 1 : 2 * a + 2],
                    msq[:, a : a + 1],
                    op=OP.subtract,
                )
            epst = s_pool.tile([16, 1], f32, tag="epst")
            nc.gpsimd.memset(epst[:], float(EPS))
            nc.scalar.activation(sd2[:], var2[:], AF.Sqrt, bias=epst[:])
            with nc.allow_low_precision(reason="f32r rounding for matmul feed"):
                nc.vector.reciprocal(grp[:, 0, 1:2], sd2[:, 0:1])
                nc.vector.reciprocal(grp[:, 1, 1:2], sd2[:, 1:2])
                nc.vector.tensor_copy(grp[:, 0, 0:1], mv[:, 0:1])
                nc.vector.tensor_copy(grp[:, 1, 0:1], mv[:, 2:3])

            # broadcast group stats to channels: pcs[:, a, 0]=mean_c, [:, a, 1]=rstd_c
            pcs = s_pool.tile([P, NCHUNK, 2], f32, tag="pcs")
            for a in range(NCHUNK):
                pcs_ps = ps_misc.tile([P, 2], f32, tag="ps_misc")
                nc.tensor.matmul(
                    pcs_ps[:], selmat[:], grp[:, a, :], start=True, stop=True
                )
                nc.vector.tensor_copy(pcs[:, a, :], pcs_ps[:])

            # per-channel affine: af[:, a, 0] = gamma*rstd, af[:, a, 1] = beta - mean*gamma*rstd
            af = s_pool.tile([P, NCHUNK, 2], f32, tag="af")
            msc = s_pool.tile([P, NCHUNK], f32, tag="msc")
            for a in range(NCHUNK):
                nc.vector.tensor_tensor(
                    af[:, a, 0:1], gam[:, a : a + 1], pcs[:, a, 1:2], op=OP.mult
                )
                nc.vector.tensor_tensor(
                    msc[:, a : a + 1], pcs[:, a, 0:1], af[:, a, 0:1], op=OP.mult
                )
                nc.vector.tensor_tensor(
                    af[:, a, 1:2], bet[:, a : a + 1], msc[:, a : a + 1], op=OP.subtract
                )
            shf_r = s_pool.tile([P, NCHUNK], f32r, tag="shf_r")
            with nc.allow_low_precision(reason="f32r rounding for matmul feed"):
                for a in range(NCHUNK):
                    nc.vector.tensor_copy(shf_r[:, a : a + 1], af[:, a, 1:2])

            # ---- fold GN into projection weights ----
            # corrected biases first (they need the unscaled weights), then
            # scale the weight rows in place.
            bq2 = s_pool.tile([P, NCHUNK], f32, tag="bq2")
            bk2 = s_pool.tile([P, NCHUNK], f32, tag="bk2")
            for w_t, b_t, b2_t in ((wq, bq, bq2), (wk, bk, bk2)):
                for b in range(NCHUNK):
                    bp = ps_misc.tile([P, 1], f32, tag="ps_misc")
                    nc.tensor.matmul(
                        bp[:],
                        w_t[:, 0, b, :].bitcast(f32),
                        af[:, 0, 1:2],
                        start=True,
                        stop=False,
                    )
                    nc.tensor.matmul(
                        bp[:],
                        w_t[:, 1, b, :].bitcast(f32),
                        af[:, 1, 1:2],
                        start=False,
                        stop=True,
                    )
                    nc.vector.tensor_tensor(
                        b2_t[:, b : b + 1], bp[:], b_t[:, b : b + 1], op=OP.add
                    )
            # v bias row: bvrow = bv + shift @ WvT, broadcast to [P, C]
            vr_ps = ps_misc.tile([1, C], f32, tag="ps_misc")
            nc.tensor.matmul(vr_ps[:], shf_r[:, 0:1], wv[:, 0, :], start=True, stop=False)
            nc.tensor.matmul(vr_ps[:], shf_r[:, 1:2], wv[:, 1, :], start=False, stop=True)
            bvrow = s_pool.tile([1, C], f32r, tag="bvrow")
            with nc.allow_low_precision(reason="f32r rounding for matmul feed"):
                nc.vector.tensor_tensor(bvrow[:], vr_ps[:], bv[:], op=OP.add)
            bvb_ps = ps_misc.tile([P, C], f32, tag="ps_misc")
            nc.tensor.matmul(bvb_ps[:], ones_r[:], bvrow[:], start=True, stop=True)
            bvb = s_pool.tile([P, C], f32, tag="bvb")
            nc.vector.tensor_copy(bvb[:], bvb_ps[:])

            # scale weight rows in place: w[c', :] *= scale[c']
            with nc.allow_low_precision(reason="f32r weights"):
                for a in range(NCHUNK):
                    nc.vector.tensor_scalar_mul(
                        wq[:, a, :, :], wq[:, a, :, :], af[:, a, 0:1]
                    )
                    nc.vector.tensor_scalar_mul(
                        wk[:, a, :, :], wk[:, a, :, :], af[:, a, 0:1]
                    )
                    nc.vector.tensor_scalar_mul(
                        wv[:, a, :], wv[:, a, :], af[:, a, 0:1]
                    )

            vt = v_pool.tile([P, NJC, C], f32r, tag="vt")
            k_t = k_pool.tile([P, NCHUNK, N], f32r, tag="k")
            q_t = q_pool.tile([P, NCHUNK, NHALF], f32r, tag="q")

            # ---- projections straight from x ----
            for s in range(8):
                xsrc = xa if s < 4 else xb
                soff = (s % 4) * BLK
                xs0 = xsrc[:, 0, soff : soff + BLK]
                xs1 = xsrc[:, 1, soff : soff + BLK]
                # q projection (first 4 strips = this core's queries)
                if s < 4:
                    for b in range(NCHUNK):
                        qp = ps_s.tile([P, BLK], f32, tag="ps_sp")
                        nc.tensor.matmul(
                            qp[:], wq[:, 0, b, :], xs0, start=True, stop=False
                        )
                        nc.tensor.matmul(
                            qp[:], wq[:, 1, b, :], xs1, start=False, stop=True
                        )
                        nc.vector.tensor_scalar_add(
                            q_t[:, b, s * BLK : (s + 1) * BLK], qp[:], bq2[:, b : b + 1]
                        )
                for b in range(NCHUNK):
                    kp = ps_s.tile([P, BLK], f32, tag="ps_sp")
                    nc.tensor.matmul(kp[:], wk[:, 0, b, :], xs0, start=True, stop=False)
                    nc.tensor.matmul(kp[:], wk[:, 1, b, :], xs1, start=False, stop=True)
                    nc.vector.tensor_scalar_add(
                        k_t[:, b, s * BLK : (s + 1) * BLK], kp[:], bk2[:, b : b + 1]
                    )
                # vT projection: strip s covers j-chunks 4s..4s+3
                for jj in range(4):
                    jc = 4 * s + jj
                    vp = ps_s.tile([P, C], f32, tag="ps_sp")
                    nc.tensor.matmul(
                        vp[:],
                        xs0[:, jj * P : (jj + 1) * P],
                        wv[:, 0, :],
                        start=True,
                        stop=False,
                    )
                    nc.tensor.matmul(
                        vp[:],
                        xs1[:, jj * P : (jj + 1) * P],
                        wv[:, 1, :],
                        start=False,
                        stop=True,
                    )
                    nc.vector.tensor_tensor(vt[:, jc, :], vp[:], bvb[:], op=OP.add)

            # ---- attention blocks ----
            # den partial accumulators: dpA fed by DVE adds (eq rows 0,1 of
            # each quarter), dpB by GpSimd adds (rows 2,3); merged per block.
            dpA = s_pool.tile([P, NBLK, BLK], f32, tag="dpA")
            dpB = s_pool.tile([P, NBLK, BLK], f32, tag="dpB")

            def den_tail(blk):
                # merge partials, cross-partition ones-matmul, copy out
                dpm = s_pool.tile([P, NBLK, BLK], f32r, tag="dpm")
                with nc.allow_low_precision(reason="f32r for ones matmul"):
                    nc.vector.tensor_tensor(
                        dpm[:, blk, :], dpA[:, blk, :], dpB[:, blk, :], op=OP.add
                    )
                den_ps = ps_misc.tile([1, BLK], f32, tag="ps_misc")
                nc.tensor.matmul(
                    den_ps[:], ones_c[:], dpm[:, blk, :], start=True, stop=True
                )
                den_sb = o_pool.tile([1, BLK], f32, tag="den_sb")
                nc.scalar.copy(den_sb[:], den_ps[:])
                nc.sync.dma_start(den_d.ap()[:, blk * BLK : (blk + 1) * BLK], den_sb[:])

            NQ = NJC // QUART
            for blk in range(NBLK):
                ib = blk * BLK
                av = ps_av.tile([P, NCHUNK, BLK], f32, tag="ps_av")
                eqs = {}
                # software pipeline: scores/exp for quarter q are emitted one
                # step ahead of AV for quarter q-1, so PE always has score
                # matmuls to run while ACT computes the exp.
                for quart in range(NQ + 1):
                    if quart < NQ:
                        eq = big16_pool.tile([P, QUART, BLK], f32r, tag="big16")
                        eqs[quart] = eq
                        for pair in range(QUART // 2):
                            sp = ps_s.tile([P, 2, BLK], f32, tag="ps_sp")
                            for u in range(2):
                                jc = quart * QUART + pair * 2 + u
                                nc.tensor.matmul(
                                    sp[:, u, :],
                                    k_t[:, 0, jc * P : (jc + 1) * P],
                                    q_t[:, 0, ib : ib + BLK],
                                    start=True,
                                    stop=False,
                                )
                                nc.tensor.matmul(
                                    sp[:, u, :],
                                    k_t[:, 1, jc * P : (jc + 1) * P],
                                    q_t[:, 1, ib : ib + BLK],
                                    start=False,
                                    stop=True,
                                )
                            nc.scalar.activation(
                                eq[:, 2 * pair : 2 * pair + 2, :],
                                sp[:],
                                AF.Exp,
                                bias=zb[:],
                                scale=SCALE,
                            )
                    if quart == 1 and blk > 0:
                        den_tail(blk - 1)
                    if quart > 0:
                        q0 = quart - 1
                        eq = eqs.pop(q0)
                        for jj in range(QUART):
                            jc = q0 * QUART + jj
                            for m in range(NCHUNK):
                                nc.tensor.matmul(
                                    av[:, m, :],
                                    vt[:, jc, m * P : (m + 1) * P],
                                    eq[:, jj, :],
                                    start=(jc == 0),
                                    stop=(jc == NJC - 1),
                                )
                        # denominator partials (contiguous adds, DVE/GpSimd)
                        if q0 == 0:
                            nc.vector.tensor_tensor(
                                dpA[:, blk, :], eq[:, 0, :], eq[:, 1, :], op=OP.add
                            )
                            nc.gpsimd.tensor_tensor(
                                dpB[:, blk, :], eq[:, 2, :], eq[:, 3, :], op=OP.add
                            )
                        else:
                            t0 = scr_pool.tile([P, BLK], f32, tag="t0")
                            nc.vector.tensor_tensor(
                                t0[:], eq[:, 0, :], eq[:, 1, :], op=OP.add
                            )
                            nc.vector.tensor_tensor(
                                dpA[:, blk, :], dpA[:, blk, :], t0[:], op=OP.add
                            )
                            t1 = scr_pool.tile([P, BLK], f32, tag="t1")
                            nc.gpsimd.tensor_tensor(
                                t1[:], eq[:, 2, :], eq[:, 3, :], op=OP.add
                            )
                            nc.gpsimd.tensor_tensor(
                                dpB[:, blk, :], dpB[:, blk, :], t1[:], op=OP.add
                            )

                # h_unnorm psum -> sbuf, then output projection (unnormalized)
                h_t = h_pool.tile([P, NCHUNK, BLK], f32r, tag="h")
                with nc.allow_low_precision(reason="f32r rounding for matmul feed"):
                    for m in range(NCHUNK):
                        nc.scalar.copy(h_t[:, m, :], av[:, m, :])

                for b in range(NCHUNK):
                    po = ps_misc.tile([P, BLK], f32, tag="ps_misc")
                    nc.tensor.matmul(
                        po[:], wo[:, 0, b, :], h_t[:, 0, :], start=True, stop=False
                    )
                    nc.tensor.matmul(
                        po[:], wo[:, 1, b, :], h_t[:, 1, :], start=False, stop=True
                    )
                    ot = o_pool.tile([P, BLK], f32, tag="o")
                    nc.vector.tensor_copy(ot[:], po[:])
                    nc.sync.dma_start(
                        out_d.ap().rearrange("a p n -> p a n")[:, b, ib : ib + BLK],
                        ot[:],
                    )

            den_tail(NBLK - 1)

    nc.compile()
    return nc


def _prep_shards(x, gamma, beta, Wq, bq, Wk, bk, Wv, bv, Wo, bo):
    xr = np.ascontiguousarray(x, dtype=np.float32).reshape(4, C, N)

    def w4(W):
        # w4[p, a, b, m] = W[b*128+m, a*128+p]
        return np.ascontiguousarray(
            np.asarray(W, np.float32).reshape(NCHUNK, P, NCHUNK, P).transpose(3, 2, 0, 1)
        )

    wv3 = np.ascontiguousarray(
        np.asarray(Wv, np.float32).reshape(C, NCHUNK, P).transpose(2, 1, 0)
    )

    def b2(v):
        return np.ascontiguousarray(np.asarray(v, np.float32).reshape(NCHUNK, P).T)

    gmat = np.zeros((P, 16), np.float32)
    for p in range(P):
        gmat[p, p // GS] = 1.0
    selmat = np.zeros((16, P), np.float32)
    for p in range(P):
        selmat[p // GS, p] = 1.0

    shared = {
        "wq": w4(Wq),
        "wk": w4(Wk),
        "wo": w4(Wo),
        "wv": wv3,
        "bq": b2(bq),
        "bk": b2(bk),
        "bv": np.ascontiguousarray(np.asarray(bv, np.float32).reshape(1, C)),
        "gamma": b2(gamma),
        "beta": b2(beta),
        "gmat": gmat,
        "selmat": selmat,
    }

    in_maps = []
    for core in range(8):
        img = core // 2
        xi = xr[img].reshape(NCHUNK, P, N)
        if core % 2 == 0:
            xa_h, xb_h = xi[:, :, :NHALF], xi[:, :, NHALF:]
        else:
            xa_h, xb_h = xi[:, :, NHALF:], xi[:, :, :NHALF]
        m = dict(shared)
        m["xa"] = np.ascontiguousarray(xa_h)
        m["xb"] = np.ascontiguousarray(xb_h)
        in_maps.append(m)
    return in_maps


def kernel(x, gamma, beta, Wq, bq, Wk, bk, Wv, bv, Wo, bo, _trace=False):
    from concourse.bass_utils import run_bass_kernel_spmd

    if "nc" not in _CACHE:
        _CACHE["nc"] = _build_program()
    nc = _CACHE["nc"]

    in_maps = _prep_shards(x, gamma, beta, Wq, bq, Wk, bk, Wv, bv, Wo, bo)
    res = run_bass_kernel_spmd(nc, in_maps, core_ids=list(range(8)), trace=_trace)
    _CACHE["last_results"] = res

    x_np = np.ascontiguousarray(x, dtype=np.float32).reshape(4, C, N)
    bo_np = np.asarray(bo, np.float32).reshape(C, 1)
    y = np.empty((4, C, N), np.float32)
    for core in range(8):
        o = res.results[core]["out"].reshape(C, NHALF)
        den = res.results[core]["den"].reshape(1, NHALF)
        img = core // 2
        lo, hi = (0, NHALF) if core % 2 == 0 else (NHALF, N)
        y[img, :, lo:hi] = x_np[img, :, lo:hi] + o / den + bo_np
    return y.reshape(4, C, 64, 64)
